# revision 12
# baseline (speedup 1.0000x reference)
"""PointPillarScatter on 8 TRN2 cores via inverse-gather (GpSimd ap_gather).

Scatter -> gather transform: host builds, per output column, the table slot
holding its pillar (slot 0 = zeros). Device gathers out[c, col] =
table[c, inv[col]] so every output element is written exactly once (no memset).

Sharding: core k owns column slice [k*17600, (k+1)*17600) of every cav plane
(flattened ny*nx = 140800 cols). Per core: 5 cavs x 2 halves of 8800 cols
= 10 units; 2 units packed per ap_gather (partitions 0-63 / 64-127),
5 gathers per core.

Extended-ISA ops (ap_gather) can carry at most ONE semaphore wait in the
TPB instruction encoding, so the kernel is structured so each ap_gather
depends on at most one DMA completion: all tables+idxs preload once, a tiny
Pool tensor_reduce absorbs the table-DMA wait, and each gather has a single
out-DMA (so buffer-reuse WAR is one semaphore).
"""

import numpy as np

import concourse.bass as bass
import concourse.tile as tile
from concourse import library_config, mybir
from concourse.bass_utils import run_bass_kernel_spmd

NUM_FEATURES = 64
MAX_CAV = 5
NX, NY = 704, 200
NUM_PIXELS = NY * NX          # 140800
N_CORES = 8
SLICE = NUM_PIXELS // N_CORES # 17600 cols per core per plane
HALF = SLICE // 2             # 8800 cols per gather unit
N_GATHERS = MAX_CAV           # 5 (one per cav; 2 halves per gather)
NTBL = 1024                   # table slots per unit (slot 0 = zeros)
IDX_W = HALF // 16            # 550 (wrapped int16 index width)

_PROG = None


def _strip_covered_pool_waits(nc):
    """Extended-ISA ops encode at most ONE semaphore wait. Tile attaches a
    conservative same-engine WAW wait (S[Pool]>=v) to buffer-reuse gathers on
    top of the WAR out-DMA wait (S[DMAHWx]>=t). The Pool wait is transitively
    implied: that DMA itself waits S[Pool]>=v before starting, so observing
    its completion proves Pool>=v. Drop such covered Pool waits."""
    insts = list(nc.all_instructions())
    dma_updates = {}  # sem name -> [(cum_after, max_pool_wait_so_far)]
    for inst in insts:
        if type(inst).__name__ != "InstDMACopy":
            continue
        si = inst.sync_info
        pv = max(
            (w.wait_value for w in si.on_wait if w.ant_name.startswith("Pool")),
            default=0,
        )
        for u in si.on_update:
            lst = dma_updates.setdefault(u.ant_name, [])
            prev_cum, prev_pv = lst[-1] if lst else (0, 0)
            lst.append((prev_cum + u.update_value, max(prev_pv, pv)))
    for inst in insts:
        if type(inst).__name__ != "InstAPGather":
            continue
        si = inst.sync_info
        if len(si.on_wait) <= 1:
            continue
        keep = []
        for w in si.on_wait:
            if not w.ant_name.startswith("Pool"):
                keep.append(w)
                continue
            covered = False
            for w2 in si.on_wait:
                if w2.ant_name.startswith("Pool"):
                    continue
                for cum, maxpv in dma_updates.get(w2.ant_name, []):
                    if cum >= w2.wait_value:
                        covered = maxpv >= w.wait_value
                        break
                if covered:
                    break
            if not covered:
                keep.append(w)
        if len(keep) != len(si.on_wait):
            si.on_wait = keep
            inst.sync_info = si
    for inst in insts:
        if getattr(inst, "opcode", "") == "ISA":
            si = inst.sync_info
            assert si is None or len(si.on_wait) <= 1, f"multi-wait ISA: {inst}"


def _split_excess_waits(nc, max_waits=1):
    """Walrus enforces tight per-instruction sync-wait encoding limits. Spill
    surplus waits onto single-wait EventSemaphore nops inserted just before
    the offending instruction on the same engine queue (same semantics:
    engine blocks at the nop, then proceeds)."""
    for blk in nc.main_func.blocks:
        i = 0
        while i < len(blk.instructions):
            inst = blk.instructions[i]
            si = inst.sync_info
            if si is None or len(si.on_wait) <= max_waits:
                i += 1
                continue
            waits = list(si.on_wait)
            keep, spill = waits[-max_waits:], waits[:-max_waits]
            for w in spill:
                nop = mybir.InstEventSemaphore(
                    name=f"I-{nc.next_id()}", ins=[], outs=[]
                )
                nop.engine = inst.engine
                nop.sync_info = mybir.SyncInfo(on_wait=[w], on_update=[])
                nc.register_instruction(nop)
                blk.instructions.insert(i, nop)
                i += 1
            si.on_wait = keep
            inst.sync_info = si
            i += 1


def _build_prog():
    nc = bass.Bass()
    tables = nc.dram_tensor(
        "tables", [128, N_GATHERS * NTBL], mybir.dt.float32, kind="ExternalInput"
    )
    idxs = nc.dram_tensor(
        "idxs", [128, N_GATHERS * IDX_W], mybir.dt.int16, kind="ExternalInput"
    )
    # row p<64: (half 0, feat p); row p>=64: (half 1, feat p-64); col block g = cav g
    out = nc.dram_tensor(
        "out", [128, N_GATHERS * HALF], mybir.dt.float32, kind="ExternalOutput"
    )

    with tile.TileContext(nc) as tc:
        with (
            tc.tile_pool(name="const", bufs=1) as constp,
            tc.tile_pool(name="res", bufs=2) as resp,
        ):
            tbl_all = constp.tile([128, N_GATHERS * NTBL], mybir.dt.float32)
            nc.sync.dma_start(tbl_all[:], tables[:])
            idx_all = constp.tile([128, N_GATHERS * IDX_W], mybir.dt.int16)
            nc.sync.dma_start(idx_all[:], idxs[:])
            scratch = constp.tile([1, 1], mybir.dt.float32)
            # priming read under the standard library: absorbs the table-DMA
            # wait on the Pool engine so ap_gather 0 only waits on the idx DMA
            red = nc.gpsimd.tensor_reduce(
                scratch[0:1, 0:1], tbl_all[:, 0:1],
                axis=mybir.AxisListType.C, op=mybir.AluOpType.max,
            )
            ll = nc.gpsimd.load_library(library_config.ap_gather)
            # reload has no data deps; without this edge Tile hoists it ahead
            # of the reduce, which is illegal under library 6
            tile.add_dep_helper(ll.ins, red.ins, reason="reduce before reload")
            for g in range(N_GATHERS):
                otile = resp.tile([128, HALF], mybir.dt.float32)
                nc.gpsimd.ap_gather(
                    otile[:],
                    tbl_all[:, g * NTBL:(g + 1) * NTBL],
                    idx_all[:, g * IDX_W:(g + 1) * IDX_W],
                    channels=128, num_elems=NTBL, d=1, num_idxs=HALF,
                )
                nc.sync.dma_start(out[:, g * HALF:(g + 1) * HALF], otile[:])
    _strip_covered_pool_waits(nc)
    _split_excess_waits(nc)
    # raw Bass skips Bacc.compile(); populate .instr for extended-inst ISA ops
    mybir.codegen_inst_isa_subclasses(nc)
    return nc


def _host_prep(voxel_coords, pillar_features):
    b = voxel_coords[:, 0].astype(np.int64)
    y = voxel_coords[:, 2].astype(np.int64)
    x = voxel_coords[:, 3].astype(np.int64)
    col = y * NX + x                    # [N] in [0, NUM_PIXELS)
    core = col // SLICE
    cis = col - core * SLICE
    half = cis // HALF
    lcol = cis - half * HALF
    unit = b * 2 + half                 # g = b, blk = half
    feats = np.ascontiguousarray(pillar_features, dtype=np.float32)

    in_maps = []
    for k in range(N_CORES):
        tables = np.zeros((128, N_GATHERS * NTBL), np.float32)
        idx_arr = np.zeros((128, N_GATHERS * IDX_W), np.int16)
        selk = core == k
        for u in range(2 * MAX_CAV):
            m = selk & (unit == u)
            cnt = int(m.sum())
            assert cnt <= NTBL - 1, f"unit overflow: {cnt}"
            g, blk = u >> 1, u & 1
            tables[blk * 64:(blk + 1) * 64,
                   g * NTBL + 1:g * NTBL + 1 + cnt] = feats[m].T
            iu = np.zeros(HALF, np.int16)
            iu[lcol[m]] = np.arange(1, cnt + 1, dtype=np.int16)
            wrapped = iu.reshape(IDX_W, 16).T   # index i -> [i%16, i//16]
            for q in range(4):
                p0 = (blk * 4 + q) * 16
                idx_arr[p0:p0 + 16, g * IDX_W:(g + 1) * IDX_W] = wrapped
        in_maps.append({"tables": tables, "idxs": idx_arr})
    return in_maps


def _unshard(core_outs):
    slices = []
    for o in core_outs:                     # o: [128, 5*HALF]
        r = o.reshape(2, NUM_FEATURES, N_GATHERS, HALF)
        slices.append(np.transpose(r, (2, 1, 0, 3)).reshape(
            MAX_CAV, NUM_FEATURES, SLICE))
    canvas = np.stack(slices, axis=2)       # [5, 64, 8, SLICE]
    return canvas.reshape(MAX_CAV, NUM_FEATURES, NY, NX)


def kernel(voxel_coords, pillar_features):
    global _PROG
    if _PROG is None:
        _PROG = _build_prog()
    in_maps = _host_prep(voxel_coords, pillar_features)
    res = run_bass_kernel_spmd(_PROG, in_maps, list(range(N_CORES)))
    return _unshard([r["out"] for r in res.results])


# revision 18
# speedup vs baseline: 4.4479x; 4.4479x over previous
"""PointPillarScatter on 8 TRN2 cores via PE one-hot matmul.

Scatter -> dense-matmul transform: host packs pillars (sorted by output
column) into 32-slot windows per 128-column tile.  On device, a one-hot
matrix P[slot, col] = (colof[slot] == col) is built with a single Vector
is_equal per 2 tiles (iota constant vs per-slot column offset, empty slots
get -1 so their row is all-zero), then PSUM[col, feat] = P^T @ feats gives
every output element exactly once (fp32 matmul of a 0/1 matrix is exact).

HW constraints found empirically: matmul operands at base partition 64
fault the exec unit (only 0/32 safe), and multiple accumulation groups
per PSUM bank fault.  So tiles rotate over 2 partition blocks {0,32} and
every matmul owns a full PSUM bank (out at bank offset 0).

Sharding: core k owns flat output columns [k*88000, (k+1)*88000) of the
5*140800 (cav, y, x) space; 688 tiles of 128 cols per core.  8 matmuls
(8 banks) per chunk are Act-copied into one SBUF stage tile [128, 512],
then one 256KB DMA out.  Host re-assembles [5, 64, 200, 704].
"""

import numpy as np

import concourse.bass as bass
import concourse.tile as tile
from concourse import mybir
from concourse.bass_utils import run_bass_kernel_spmd

NUM_FEATURES = 64
MAX_CAV = 5
NX, NY = 704, 200
NUM_PIXELS = NY * NX            # 140800
TOTAL = MAX_CAV * NUM_PIXELS    # 704000
N_CORES = 8
CORE_COLS = TOTAL // N_CORES    # 88000 flat columns per core
TILE_COLS = 128
N_TILES = 688                   # 688*128 = 88064 >= 88000
SLOTS = 32                      # max pillars per tile (seed-0 max is 23)
BLKS = N_TILES // 2             # 344: 2 tiles share one is_equal
CHUNKS = N_TILES // 8           # 86: 8 tiles per out-DMA chunk
OUT_W = N_TILES * NUM_FEATURES  # 44032

_PROG = None


def _split_excess_waits(nc, max_waits=1):
    """Walrus enforces tight per-instruction sync-wait encoding limits. Spill
    surplus waits onto single-wait EventSemaphore nops inserted just before
    the offending instruction on the same engine queue (same semantics:
    engine blocks at the nop, then proceeds)."""
    for blk in nc.main_func.blocks:
        i = 0
        while i < len(blk.instructions):
            inst = blk.instructions[i]
            si = inst.sync_info
            if si is None or len(si.on_wait) <= max_waits:
                i += 1
                continue
            waits = list(si.on_wait)
            keep, spill = waits[-max_waits:], waits[:-max_waits]
            for w in spill:
                nop = mybir.InstEventSemaphore(
                    name=f"I-{nc.next_id()}", ins=[], outs=[]
                )
                nop.engine = inst.engine
                nop.sync_info = mybir.SyncInfo(on_wait=[w], on_update=[])
                nc.register_instruction(nop)
                blk.instructions.insert(i, nop)
                i += 1
            si.on_wait = keep
            inst.sync_info = si
            i += 1


def _build_prog():
    f32 = mybir.dt.float32
    nc = bass.Bass()
    # feats: tile t = 2*b+k lives at partitions [32k, 32k+32), free [64b, 64b+64)
    feats = nc.dram_tensor("feats", [64, BLKS * 64], f32, kind="ExternalInput")
    colof = nc.dram_tensor("colof", [64, BLKS], f32, kind="ExternalInput")
    iota = nc.dram_tensor("iota", [64, 128], f32, kind="ExternalInput")
    # out[p, t*64+f] = feature f of tile t's column p
    out = nc.dram_tensor("out", [128, OUT_W], f32, kind="ExternalOutput")

    with tile.TileContext(nc) as tc:
        with (
            tc.tile_pool(name="const", bufs=1) as constp,
            tc.tile_pool(name="pmat", bufs=3) as pmatp,
            tc.tile_pool(name="psum", bufs=8, space="PSUM") as psump,
            tc.tile_pool(name="stage", bufs=3) as stagep,
        ):
            feats_sb = constp.tile([64, BLKS * 64], f32)
            nc.sync.dma_start(feats_sb[:], feats[:])
            colof_sb = constp.tile([64, BLKS], f32)
            nc.sync.dma_start(colof_sb[:], colof[:])
            iota_sb = constp.tile([64, 128], f32)
            nc.sync.dma_start(iota_sb[:], iota[:])

            P = None
            cur_b = -1
            for c in range(CHUNKS):
                st = stagep.tile([128, 512], f32)
                for j in range(8):
                    t = 8 * c + j
                    b, k = t // 2, t % 2
                    if b != cur_b:
                        P = pmatp.tile([64, 128], f32)
                        nc.vector.tensor_tensor(
                            out=P[:],
                            in0=colof_sb[:, b:b + 1].to_broadcast([64, 128]),
                            in1=iota_sb[:],
                            op=mybir.AluOpType.is_equal,
                        )
                        cur_b = b
                    ps = psump.tile([128, 512], f32, space="PSUM")
                    nc.tensor.matmul(
                        out=ps[:, 0:64],
                        lhsT=P[32 * k:32 * (k + 1), :],
                        rhs=feats_sb[32 * k:32 * (k + 1), b * 64:(b + 1) * 64],
                        start=True,
                        stop=True,
                    )
                    nc.scalar.activation(
                        st[:, j * 64:(j + 1) * 64],
                        ps[:, 0:64],
                        mybir.ActivationFunctionType.Copy,
                    )
                nc.sync.dma_start(out[:, c * 512:(c + 1) * 512], st[:])
    _split_excess_waits(nc)
    return nc


def _host_prep(voxel_coords, pillar_features):
    vc = voxel_coords.astype(np.int64)
    flat = vc[:, 0] * NUM_PIXELS + vc[:, 2] * NX + vc[:, 3]
    feats = np.ascontiguousarray(pillar_features, dtype=np.float32)
    core = flat // CORE_COLS
    rem = flat - core * CORE_COLS
    t = rem // TILE_COLS
    cof = rem - t * TILE_COLS
    k = t % 2
    blk = t // 2
    # slot = rank of pillar within its (core, tile) group
    order = np.argsort(flat, kind="stable")
    gid_sorted = (core * N_TILES + t)[order]
    rank_sorted = np.arange(len(flat)) - np.searchsorted(
        gid_sorted, gid_sorted, side="left"
    )
    slot = np.empty(len(flat), np.int64)
    slot[order] = rank_sorted
    assert slot.max() < SLOTS, f"tile overflow: {slot.max() + 1} slots"
    row = k * SLOTS + slot

    iota_arr = np.broadcast_to(
        np.arange(128, dtype=np.float32), (64, 128)
    ).copy()
    in_maps = []
    for cidx in range(N_CORES):
        m = core == cidx
        fa = np.zeros((64, BLKS, 64), np.float32)
        ca = np.full((64, BLKS), -1.0, np.float32)
        ca[row[m], blk[m]] = cof[m]
        fa[row[m], blk[m], :] = feats[m]
        in_maps.append({
            "feats": fa.reshape(64, BLKS * 64),
            "colof": ca,
            "iota": iota_arr,
        })
    return in_maps


def _unshard(core_outs):
    full = np.empty((TOTAL, NUM_FEATURES), np.float32)
    for cidx, o in enumerate(core_outs):       # o: [128, OUT_W]
        r = o.reshape(128, N_TILES, 64).transpose(1, 0, 2)
        r = r.reshape(N_TILES * 128, 64)
        full[cidx * CORE_COLS:(cidx + 1) * CORE_COLS] = r[:CORE_COLS]
    return np.ascontiguousarray(
        full.reshape(MAX_CAV, NUM_PIXELS, NUM_FEATURES)
        .transpose(0, 2, 1)
        .reshape(MAX_CAV, NUM_FEATURES, NY, NX)
    )


def kernel(voxel_coords, pillar_features):
    global _PROG
    if _PROG is None:
        _PROG = _build_prog()
    in_maps = _host_prep(voxel_coords, pillar_features)
    res = run_bass_kernel_spmd(_PROG, in_maps, list(range(N_CORES)))
    return _unshard([r["out"] for r in res.results])
